# revision 4
# baseline (speedup 1.0000x reference)
"""Bass/Trainium2 kernel for nn_NodeAttention (PyG TransformerConv-style
edge attention + per-graph scatter softmax), data-parallel over graphs on
8 NeuronCores.

Contract: kernel(**inputs) takes the FULL unsharded inputs (as produced by
reference.setup_inputs()) and returns the full (alpha_map, h_weighted).

Sharding strategy (hardcoded): 512 graphs of 116 nodes -> 64 graphs/core,
processed as 8 blocks x 8 graphs. Wq/Wk/Wv/Ws replicated. Edge softmax and
graph softmax are both graph-local so there is no cross-core communication.

Algorithm per graph g (116 nodes, 8 incoming edges per node):
  Dense scores St[j,i] = k_j . q_i / sqrt(116)  (PE matmul, fp16 in / f32 acc)
  W[j,i] = C[j,i] * exp(St[j,i])  where C counts edges src j -> dst i
           (C=0 masks non-edges; counts handle duplicate edges; the softmax
            max-subtraction is skipped -- scores are ~N(0,1) so exp is safe)
  [msg | denom] = W^T @ [V | 1]   (one matmul, ones-column trick)
  hproj = msg / denom + h @ Ws    (fused DVE op, emits row-sum for phase 2)
  Graph softmax over nodes: transpose the per-block e-matrix [116,8] with the
  PE so the node axis lands on the free dim, exp+sum on ACT, normalize, and
  transpose alpha back for the per-node scaling of hproj.
"""

import math
import sys

sys.path.insert(0, "/opt/trn_rl_repo")

import numpy as np

from concourse import bacc, mybir, tile
from concourse.bass_utils import run_bass_kernel_spmd
from concourse.masks import make_identity

# Problem geometry (fixed by the nn_NodeAttention_66365834658168 spec)
B = 512           # graphs
NPG = 116         # nodes per graph
HEADS = 2
DH = 116          # head dim
HID = 256
DOUT = HEADS * DH  # 232
DEG = 8
N = B * NPG       # 59392
N_CORES = 8
GPC = B // N_CORES      # 64 graphs per core
NPC = GPC * NPG         # 7424 nodes per core
BLK = 8                 # graphs per block
NBLK = GPC // BLK       # 8 blocks per core
BN = BLK * NPG          # 928 nodes per block

F16 = mybir.dt.float16
F32 = mybir.dt.float32
AF = mybir.ActivationFunctionType
OP = mybir.AluOpType

_CACHE = {}


def _build_module():
    """Build + compile the per-core Bass module (same program on all cores)."""
    nc = bacc.Bacc("TRN2", target_bir_lowering=False, debug=False,
                   num_devices=N_CORES)

    ht_d = nc.dram_tensor("ht", [2, 128, NPC], F16, kind="ExternalInput")
    ct_d = nc.dram_tensor("ct", [NBLK, NPG, BN], F16, kind="ExternalInput")
    wq_d = nc.dram_tensor("wq", [2, 128, DOUT], F16, kind="ExternalInput")
    wk_d = nc.dram_tensor("wk", [2, 128, DOUT], F16, kind="ExternalInput")
    wvs_d = nc.dram_tensor("wvs", [2, 128, 2 * DOUT], F16, kind="ExternalInput")
    alpha_d = nc.dram_tensor("alpha", [GPC, NPG], F32, kind="ExternalOutput")
    hw_d = nc.dram_tensor("hw", [NPC, DOUT], F32, kind="ExternalOutput")

    RT = 1.0 / math.sqrt(float(DH))

    with tile.TileContext(nc) as tc:
        with (
            tc.tile_pool(name="const", bufs=1) as constp,
            tc.tile_pool(name="wpool", bufs=1) as wpool,
            tc.tile_pool(name="htp", bufs=2) as htp,
            tc.tile_pool(name="ctp", bufs=2) as ctp,
            tc.tile_pool(name="qkp", bufs=2) as qkp,
            tc.tile_pool(name="vp", bufs=4) as vp,
            tc.tile_pool(name="attp", bufs=4) as attp,
            tc.tile_pool(name="smallp", bufs=24) as smallp,
            tc.tile_pool(name="hwp", bufs=2) as hwp,
            tc.tile_pool(name="ph2", bufs=2) as ph2p,
            tc.tile_pool(name="pj_ps", bufs=2, space="PSUM") as pj_ps,
            tc.tile_pool(name="vs_ps", bufs=2, space="PSUM") as vs_ps,
            tc.tile_pool(name="at_ps", bufs=2, space="PSUM") as at_ps,
            tc.tile_pool(name="t_ps", bufs=1, space="PSUM") as t_ps,
        ):
            idt = constp.tile([128, 128], F32)
            make_identity(nc, idt[:])

            wq_sb = wpool.tile([128, 2 * DOUT], F16, tag="wq")
            nc.sync.dma_start(
                wq_sb[:].rearrange("p (k m) -> p k m", k=2),
                wq_d.ap().rearrange("k p m -> p k m"),
            )
            wk_sb = wpool.tile([128, 2 * DOUT], F16, tag="wk")
            nc.sync.dma_start(
                wk_sb[:].rearrange("p (k m) -> p k m", k=2),
                wk_d.ap().rearrange("k p m -> p k m"),
            )
            wvs_sb = wpool.tile([128, 2 * 2 * DOUT], F16, tag="wvs")
            nc.sync.dma_start(
                wvs_sb[:].rearrange("p (k m) -> p k m", k=2),
                wvs_d.ap().rearrange("k p m -> p k m"),
            )

            for b in range(NBLK):
                # ---- loads for this block ----
                ht_sb = htp.tile([128, 2 * BN], F16, tag="ht")
                nc.sync.dma_start(
                    ht_sb[:].rearrange("p (k n) -> p k n", k=2),
                    ht_d.ap()[:, :, b * BN:(b + 1) * BN].rearrange(
                        "k p n -> p k n"),
                )
                ct_sb = ctp.tile([NPG, BN], F16, tag="ct")
                nc.sync.dma_start(ct_sb[:], ct_d.ap()[b])

                # ---- Q/K projections (transposed layout [head_d, node]) ----
                qt_sb = qkp.tile([128, 2 * BN], F16, tag="qt")
                kt_sb = qkp.tile([128, 2 * BN], F16, tag="kt")
                for wsb, tsb, eng in ((wq_sb, qt_sb, "v"), (wk_sb, kt_sb, "s")):
                    for h in range(HEADS):
                        for ch in range(2):
                            ps = pj_ps.tile([DH, 464], F32, tag="pj")
                            for k in range(2):
                                nc.tensor.matmul(
                                    ps[:],
                                    lhsT=wsb[:, k * DOUT + h * DH:
                                             k * DOUT + h * DH + DH],
                                    rhs=ht_sb[:, k * BN + ch * 464:
                                              k * BN + ch * 464 + 464],
                                    start=(k == 0), stop=(k == 1),
                                )
                            dst = tsb[:DH, h * BN + ch * 464:
                                      h * BN + ch * 464 + 464]
                            if eng == "v":
                                nc.vector.tensor_copy(dst, ps[:])
                            else:
                                nc.scalar.copy(dst, ps[:])

                hwb = hwp.tile([NPG, BLK * DOUT], F32, tag="hw")
                e_blk = ph2p.tile([NPG, BLK], F32, tag="eblk")

                for gib in range(BLK):
                    g0 = gib * NPG
                    # ---- V and skip (h@Ws) projections, natural layout ----
                    vs = vs_ps.tile([NPG, 2 * DOUT], F32, tag="vs")
                    for k in range(2):
                        nc.tensor.matmul(
                            vs[:],
                            lhsT=ht_sb[:, k * BN + g0:k * BN + g0 + NPG],
                            rhs=wvs_sb[:, k * 2 * DOUT:(k + 1) * 2 * DOUT],
                            start=(k == 0), stop=(k == 1),
                        )
                    # V packed per head with a ones column: [V_h | 1] x2
                    v_sb = vp.tile([NPG, 2 * (DH + 1)], F16, tag="v")
                    v3 = v_sb[:].rearrange("p (h x) -> p h x", h=2)
                    nc.scalar.copy(
                        v3[:, :, 0:DH],
                        vs[:, 0:DOUT].rearrange("p (h d) -> p h d", h=2),
                    )
                    nc.gpsimd.memset(v3[:, :, DH:DH + 1], 1.0)
                    skip_sb = vp.tile([NPG, DOUT], F32, tag="skip")
                    nc.scalar.copy(skip_sb[:], vs[:, DOUT:2 * DOUT])

                    e_hs = []
                    for h in range(HEADS):
                        st = at_ps.tile([NPG, DH + 1], F32, tag="at")
                        nc.tensor.matmul(
                            st[:, :NPG],
                            lhsT=kt_sb[:DH, h * BN + g0:h * BN + g0 + NPG],
                            rhs=qt_sb[:DH, h * BN + g0:h * BN + g0 + NPG],
                            start=True, stop=True,
                        )
                        ex = attp.tile([NPG, NPG], F16, tag="ex")
                        nc.scalar.activation(ex[:], st[:, :NPG], AF.Exp,
                                             scale=RT)
                        wt = attp.tile([NPG, NPG], F16, tag="wt")
                        nc.gpsimd.tensor_mul(wt[:], ex[:],
                                             ct_sb[:, g0:g0 + NPG])
                        mg = at_ps.tile([NPG, DH + 1], F32, tag="at")
                        nc.tensor.matmul(
                            mg[:],
                            lhsT=wt[:],
                            rhs=v_sb[:, h * (DH + 1):(h + 1) * (DH + 1)],
                            start=True, stop=True,
                        )
                        rec = smallp.tile([NPG, 1], F32, tag="rec")
                        nc.vector.reciprocal(rec[:], mg[:, DH:DH + 1])
                        e_h = smallp.tile([NPG, 1], F32, tag="eh")
                        nc.vector.scalar_tensor_tensor(
                            out=hwb[:, gib * DOUT + h * DH:
                                    gib * DOUT + h * DH + DH],
                            in0=mg[:, 0:DH],
                            scalar=rec[:],
                            in1=skip_sb[:, h * DH:(h + 1) * DH],
                            op0=OP.mult, op1=OP.add,
                            accum_out=e_h[:],
                        )
                        e_hs.append(e_h)
                    nc.gpsimd.tensor_add(e_blk[:, gib:gib + 1],
                                         e_hs[0][:], e_hs[1][:])

                # ---- phase 2: per-graph softmax over the 116 nodes ----
                eT = t_ps.tile([BLK, NPG], F32, tag="tp")
                nc.tensor.transpose(out=eT[:], in_=e_blk[:],
                                    identity=idt[:NPG, :NPG])
                ee_rows = ph2p.tile([BLK, NPG], F32, tag="ee")
                es = ph2p.tile([BLK, 1], F32, tag="es")
                nc.scalar.activation(ee_rows[:], eT[:], AF.Exp,
                                     scale=1.0 / DOUT, accum_out=es[:])
                rs = ph2p.tile([BLK, 1], F32, tag="rs")
                nc.vector.reciprocal(rs[:], es[:])
                al_rows = ph2p.tile([BLK, NPG], F32, tag="al")
                nc.vector.tensor_scalar_mul(al_rows[:], ee_rows[:], rs[:])
                nc.sync.dma_start(alpha_d.ap()[b * BLK:(b + 1) * BLK, :],
                                  al_rows[:])
                alT = t_ps.tile([NPG, BLK], F32, tag="tp")
                nc.tensor.transpose(out=alT[:], in_=al_rows[:],
                                    identity=idt[:BLK, :BLK])
                al_cols = ph2p.tile([NPG, BLK], F32, tag="alc")
                nc.vector.tensor_copy(al_cols[:], alT[:])
                for gib in range(BLK):
                    sl = hwb[:, gib * DOUT:(gib + 1) * DOUT]
                    nc.gpsimd.tensor_scalar_mul(sl, sl,
                                                al_cols[:, gib:gib + 1])
                nc.sync.dma_start(
                    hw_d.ap()[b * BN:(b + 1) * BN, :].rearrange(
                        "(g p) d -> p g d", p=NPG),
                    hwb[:].rearrange("p (g d) -> p g d", g=BLK),
                )

    nc.compile()
    return nc


def _preprocess(h_flat, edge_index, Wq, Wk, Wv, Ws):
    h = np.asarray(h_flat, dtype=np.float32)
    hT16 = np.ascontiguousarray(h.T).astype(np.float16)  # [256, N]

    ei = np.asarray(edge_index, dtype=np.int64)
    src, dst = ei[0], ei[1]
    g = dst // NPG
    i_off = dst - g * NPG
    j_off = src - g * NPG
    cnt = np.bincount(((g * NPG + j_off) * NPG + i_off),
                      minlength=B * NPG * NPG).astype(np.float16)
    cnt = cnt.reshape(B, NPG, NPG)  # [graph, src j, dst i]

    wq16 = np.ascontiguousarray(
        np.asarray(Wq, np.float32).astype(np.float16).reshape(2, 128, DOUT))
    wk16 = np.ascontiguousarray(
        np.asarray(Wk, np.float32).astype(np.float16).reshape(2, 128, DOUT))
    wvs16 = np.ascontiguousarray(
        np.concatenate([np.asarray(Wv, np.float32), np.asarray(Ws, np.float32)],
                       axis=1).astype(np.float16).reshape(2, 128, 2 * DOUT))

    in_maps = []
    for c in range(N_CORES):
        ht_c = np.ascontiguousarray(
            hT16[:, c * NPC:(c + 1) * NPC]).reshape(2, 128, NPC)
        ct_c = np.ascontiguousarray(
            cnt[c * GPC:(c + 1) * GPC]
            .reshape(NBLK, BLK, NPG, NPG)      # [blk, gib, j, i]
            .transpose(0, 2, 1, 3)             # [blk, j, gib, i]
            .reshape(NBLK, NPG, BN))
        in_maps.append({"ht": ht_c, "ct": ct_c,
                        "wq": wq16, "wk": wk16, "wvs": wvs16})
    return in_maps


def kernel(h_flat, edge_index, batch_index, Wq, Wk, Wv, Ws, **_ignored):
    if "nc" not in _CACHE:
        _CACHE["nc"] = _build_module()
    nc = _CACHE["nc"]

    in_maps = _preprocess(h_flat, edge_index, Wq, Wk, Wv, Ws)
    res = run_bass_kernel_spmd(nc, in_maps, list(range(N_CORES)))

    alpha_map = np.concatenate(
        [res.results[c]["alpha"] for c in range(N_CORES)], axis=0)
    h_weighted = np.concatenate(
        [res.results[c]["hw"] for c in range(N_CORES)], axis=0)
    return alpha_map.reshape(B, NPG), h_weighted.reshape(N, DOUT)


# revision 7
# speedup vs baseline: 2.7773x; 2.7773x over previous
"""Bass/Trainium2 kernel for nn_NodeAttention (PyG TransformerConv-style
edge attention + per-graph scatter softmax), data-parallel over graphs on
8 NeuronCores.

Contract: kernel(**inputs) takes the FULL unsharded inputs (as produced by
reference.setup_inputs()) and returns the full (alpha_map, h_weighted).

Sharding strategy (hardcoded): 512 graphs of 116 nodes -> 64 graphs/core,
processed as 8 blocks x 8 graphs. Wq/Wk/Wv/Ws replicated. Edge softmax and
graph softmax are both graph-local so there is no cross-core communication.

Algorithm per graph g (116 nodes, 8 incoming edges per node):
  Dense scores St[j,i] = k_j . q_i / sqrt(116)  (PE matmul, fp16 in / f32 acc)
  W[j,i] = C[j,i] * exp(St[j,i])  where C counts edges src j -> dst i
           (C=0 masks non-edges; counts handle duplicate edges; the softmax
            max-subtraction is skipped -- scores are ~N(0,1) so exp is safe)
  [msg | denom] = W^T @ [V | 1]   (one matmul per head, ones-column trick)
  hproj = msg / denom + h @ Ws    (fused DVE op, emits row-sum for phase 2)
  Graph softmax over nodes: transpose the per-block e-matrix [116,8] with the
  PE so the node axis lands on the free dim, exp+sum on ACT, normalize, and
  transpose alpha back for the per-node scaling of hproj.

Both heads share one PSUM tile for scores/messages so exp / mask-multiply /
reciprocal run once per graph at [116,232] granularity (ACT per-op overhead
is ~180ns; DVE fp16 SBUF ops hit the 2x perf mode).

h_weighted is written to DRAM in a block-contiguous scratch layout
[8, 116, 8*232] (one 7.4KB run per partition per block -> ~116 DMA
descriptors instead of 928) and transposed to [N, 232] on the host.
"""

import math
import sys

sys.path.insert(0, "/opt/trn_rl_repo")

import numpy as np

from concourse import bacc, mybir, tile
from concourse.bass_utils import run_bass_kernel_spmd
from concourse.masks import make_identity

# Problem geometry (fixed by the nn_NodeAttention_66365834658168 spec)
B = 512           # graphs
NPG = 116         # nodes per graph
HEADS = 2
DH = 116          # head dim
HID = 256
DOUT = HEADS * DH  # 232
DEG = 8
N = B * NPG       # 59392
N_CORES = 8
GPC = B // N_CORES      # 64 graphs per core
NPC = GPC * NPG         # 7424 nodes per core
BLK = 8                 # graphs per block
NBLK = GPC // BLK       # 8 blocks per core
BN = BLK * NPG          # 928 nodes per block

F16 = mybir.dt.float16
F32 = mybir.dt.float32
AF = mybir.ActivationFunctionType
OP = mybir.AluOpType

_CACHE = {}


def _build_module():
    """Build + compile the per-core Bass module (same program on all cores)."""
    nc = bacc.Bacc("TRN2", target_bir_lowering=False, debug=False,
                   num_devices=N_CORES)

    ht_d = nc.dram_tensor("ht", [2, 128, NPC], F16, kind="ExternalInput")
    ct_d = nc.dram_tensor("ct", [NBLK, NPG, BLK * DOUT], F16,
                          kind="ExternalInput")
    wq_d = nc.dram_tensor("wq", [2, 128, DOUT], F16, kind="ExternalInput")
    wk_d = nc.dram_tensor("wk", [2, 128, DOUT], F16, kind="ExternalInput")
    wvs_d = nc.dram_tensor("wvs", [2, 128, 2 * DOUT], F16, kind="ExternalInput")
    alpha_d = nc.dram_tensor("alpha", [GPC, NPG], F32, kind="ExternalOutput")
    # block-contiguous scratch layout; host transposes to [N, 232]
    hw_d = nc.dram_tensor("hw", [NBLK, NPG, BLK * DOUT], F32,
                          kind="ExternalOutput")

    RT = 1.0 / math.sqrt(float(DH))

    with tile.TileContext(nc) as tc:
        with (
            tc.tile_pool(name="const", bufs=1) as constp,
            tc.tile_pool(name="wpool", bufs=1) as wpool,
            tc.tile_pool(name="htp", bufs=2) as htp,
            tc.tile_pool(name="ctp", bufs=2) as ctp,
            tc.tile_pool(name="qkp", bufs=2) as qkp,
            tc.tile_pool(name="vp", bufs=4) as vp,
            tc.tile_pool(name="attp", bufs=4) as attp,
            tc.tile_pool(name="smallp", bufs=24) as smallp,
            tc.tile_pool(name="hwp", bufs=2) as hwp,
            tc.tile_pool(name="ph2", bufs=2) as ph2p,
            tc.tile_pool(name="pj_ps", bufs=2, space="PSUM") as pj_ps,
            tc.tile_pool(name="vs_ps", bufs=2, space="PSUM") as vs_ps,
            tc.tile_pool(name="at_ps", bufs=2, space="PSUM") as at_ps,
        ):
            idt = constp.tile([128, 128], F32)
            make_identity(nc, idt[:])

            wq_sb = wpool.tile([128, 2 * DOUT], F16, tag="wq")
            nc.sync.dma_start(
                wq_sb[:].rearrange("p (k m) -> p k m", k=2),
                wq_d.ap().rearrange("k p m -> p k m"),
            )
            wk_sb = wpool.tile([128, 2 * DOUT], F16, tag="wk")
            nc.sync.dma_start(
                wk_sb[:].rearrange("p (k m) -> p k m", k=2),
                wk_d.ap().rearrange("k p m -> p k m"),
            )
            wvs_sb = wpool.tile([128, 2 * 2 * DOUT], F16, tag="wvs")
            nc.sync.dma_start(
                wvs_sb[:].rearrange("p (k m) -> p k m", k=2),
                wvs_d.ap().rearrange("k p m -> p k m"),
            )

            for b in range(NBLK):
                # ---- loads for this block ----
                ht_sb = htp.tile([128, 2 * BN], F16, tag="ht")
                nc.sync.dma_start(
                    ht_sb[:].rearrange("p (k n) -> p k n", k=2),
                    ht_d.ap()[:, :, b * BN:(b + 1) * BN].rearrange(
                        "k p n -> p k n"),
                )
                ct_sb = ctp.tile([NPG, BLK * DOUT], F16, tag="ct")
                nc.sync.dma_start(ct_sb[:], ct_d.ap()[b])

                # ---- Q/K projections (transposed layout [head_d, node]) ----
                qt_sb = qkp.tile([128, 2 * BN], F16, tag="qt")
                kt_sb = qkp.tile([128, 2 * BN], F16, tag="kt")
                for wsb, tsb, eng in ((wq_sb, qt_sb, "v"), (wk_sb, kt_sb, "s")):
                    for h in range(HEADS):
                        for ch in range(2):
                            ps = pj_ps.tile([DH, 464], F32, tag="pj")
                            for k in range(2):
                                nc.tensor.matmul(
                                    ps[:],
                                    lhsT=wsb[:, k * DOUT + h * DH:
                                             k * DOUT + h * DH + DH],
                                    rhs=ht_sb[:, k * BN + ch * 464:
                                              k * BN + ch * 464 + 464],
                                    start=(k == 0), stop=(k == 1),
                                )
                            dst = tsb[:DH, h * BN + ch * 464:
                                      h * BN + ch * 464 + 464]
                            if eng == "v":
                                nc.vector.tensor_copy(dst, ps[:])
                            else:
                                nc.scalar.copy(dst, ps[:])

                hwb = hwp.tile([NPG, BLK * DOUT], F32, tag="hw")
                e_blk = ph2p.tile([NPG, BLK], F32, tag="eblk")

                for gib in range(BLK):
                    g0 = gib * NPG
                    # ---- V and skip (h@Ws) projections, natural layout ----
                    vs = vs_ps.tile([NPG, 2 * DOUT], F32, tag="vs")
                    for k in range(2):
                        nc.tensor.matmul(
                            vs[:],
                            lhsT=ht_sb[:, k * BN + g0:k * BN + g0 + NPG],
                            rhs=wvs_sb[:, k * 2 * DOUT:(k + 1) * 2 * DOUT],
                            start=(k == 0), stop=(k == 1),
                        )
                    # V packed per head with a ones column: [V_h | 1] x2
                    v_sb = vp.tile([NPG, 2 * (DH + 1)], F16, tag="v")
                    v3 = v_sb[:].rearrange("p (h x) -> p h x", h=2)
                    nc.scalar.copy(
                        v3[:, :, 0:DH],
                        vs[:, 0:DOUT].rearrange("p (h d) -> p h d", h=2),
                    )
                    nc.gpsimd.memset(v3[:, :, DH:DH + 1], 1.0)
                    skip_sb = vp.tile([NPG, DOUT], F32, tag="skip")
                    nc.scalar.copy(skip_sb[:], vs[:, DOUT:2 * DOUT])

                    # ---- attention: both heads share one PSUM tile ----
                    st = at_ps.tile([NPG, DOUT], F32, tag="st")
                    for h in range(HEADS):
                        nc.tensor.matmul(
                            st[:, h * DH:(h + 1) * DH],
                            lhsT=kt_sb[:DH, h * BN + g0:h * BN + g0 + NPG],
                            rhs=qt_sb[:DH, h * BN + g0:h * BN + g0 + NPG],
                            start=True, stop=True,
                        )
                    ex = attp.tile([NPG, DOUT], F16, tag="ex")
                    nc.scalar.activation(ex[:], st[:], AF.Exp, scale=RT)
                    wt = attp.tile([NPG, DOUT], F16, tag="wt")
                    nc.vector.tensor_mul(wt[:], ex[:],
                                         ct_sb[:, gib * DOUT:(gib + 1) * DOUT])
                    mg = at_ps.tile([NPG, 2 * (DH + 1)], F32, tag="mg")
                    for h in range(HEADS):
                        nc.tensor.matmul(
                            mg[:, h * (DH + 1):(h + 1) * (DH + 1)],
                            lhsT=wt[:, h * DH:(h + 1) * DH],
                            rhs=v_sb[:, h * (DH + 1):(h + 1) * (DH + 1)],
                            start=True, stop=True,
                        )
                    mg3 = mg[:].rearrange("p (h x) -> p h x", h=2)
                    rec = smallp.tile([NPG, 2], F32, tag="rec")
                    nc.vector.reciprocal(rec[:], mg3[:, :, DH:DH + 1])
                    e_hs = []
                    for h in range(HEADS):
                        e_h = smallp.tile([NPG, 1], F32, tag="eh")
                        nc.vector.scalar_tensor_tensor(
                            out=hwb[:, gib * DOUT + h * DH:
                                    gib * DOUT + h * DH + DH],
                            in0=mg3[:, h, 0:DH],
                            scalar=rec[:, h:h + 1],
                            in1=skip_sb[:, h * DH:(h + 1) * DH],
                            op0=OP.mult, op1=OP.add,
                            accum_out=e_h[:],
                        )
                        e_hs.append(e_h)
                    nc.gpsimd.tensor_add(e_blk[:, gib:gib + 1],
                                         e_hs[0][:], e_hs[1][:])

                # ---- phase 2: per-graph softmax over the 116 nodes ----
                eT = at_ps.tile([BLK, NPG], F32, tag="st")
                nc.tensor.transpose(out=eT[:], in_=e_blk[:],
                                    identity=idt[:NPG, :NPG])
                ee_rows = ph2p.tile([BLK, NPG], F32, tag="ee")
                es = ph2p.tile([BLK, 1], F32, tag="es")
                nc.scalar.activation(ee_rows[:], eT[:], AF.Exp,
                                     scale=1.0 / DOUT, accum_out=es[:])
                rs = ph2p.tile([BLK, 1], F32, tag="rs")
                nc.vector.reciprocal(rs[:], es[:])
                al_rows = ph2p.tile([BLK, NPG], F32, tag="al")
                nc.vector.tensor_scalar_mul(al_rows[:], ee_rows[:], rs[:])
                nc.sync.dma_start(alpha_d.ap()[b * BLK:(b + 1) * BLK, :],
                                  al_rows[:])
                alT = at_ps.tile([NPG, BLK], F32, tag="st")
                nc.tensor.transpose(out=alT[:], in_=al_rows[:],
                                    identity=idt[:BLK, :BLK])
                al_cols = ph2p.tile([NPG, BLK], F32, tag="alc")
                nc.vector.tensor_copy(al_cols[:], alT[:])
                for gib in range(BLK):
                    sl = hwb[:, gib * DOUT:(gib + 1) * DOUT]
                    nc.vector.tensor_scalar_mul(sl, sl,
                                                al_cols[:, gib:gib + 1])
                nc.sync.dma_start(hw_d.ap()[b], hwb[:])

    nc.compile()
    return nc


def _preprocess(h_flat, edge_index, Wq, Wk, Wv, Ws):
    h = np.asarray(h_flat, dtype=np.float32)
    hT16 = np.ascontiguousarray(h.T).astype(np.float16)  # [256, N]

    ei = np.asarray(edge_index, dtype=np.int64)
    src, dst = ei[0], ei[1]
    g = dst // NPG
    i_off = dst - g * NPG
    j_off = src - g * NPG
    cnt = np.bincount(((g * NPG + j_off) * NPG + i_off),
                      minlength=B * NPG * NPG).astype(np.float16)
    cnt = cnt.reshape(B, NPG, NPG)  # [graph, src j, dst i]

    wq16 = np.ascontiguousarray(
        np.asarray(Wq, np.float32).astype(np.float16).reshape(2, 128, DOUT))
    wk16 = np.ascontiguousarray(
        np.asarray(Wk, np.float32).astype(np.float16).reshape(2, 128, DOUT))
    wvs16 = np.ascontiguousarray(
        np.concatenate([np.asarray(Wv, np.float32), np.asarray(Ws, np.float32)],
                       axis=1).astype(np.float16).reshape(2, 128, 2 * DOUT))

    in_maps = []
    for c in range(N_CORES):
        ht_c = np.ascontiguousarray(
            hT16[:, c * NPC:(c + 1) * NPC]).reshape(2, 128, NPC)
        cc = cnt[c * GPC:(c + 1) * GPC].reshape(NBLK, BLK, NPG, NPG)
        # [blk, gib, j, i] -> [blk, j, gib, h, i] with the head axis duplicated
        cc = np.broadcast_to(cc.transpose(0, 2, 1, 3)[:, :, :, None, :],
                             (NBLK, NPG, BLK, HEADS, NPG))
        ct_c = np.ascontiguousarray(cc).reshape(NBLK, NPG, BLK * DOUT)
        in_maps.append({"ht": ht_c, "ct": ct_c,
                        "wq": wq16, "wk": wk16, "wvs": wvs16})
    return in_maps


def kernel(h_flat, edge_index, batch_index, Wq, Wk, Wv, Ws, **_ignored):
    if "nc" not in _CACHE:
        _CACHE["nc"] = _build_module()
    nc = _CACHE["nc"]

    in_maps = _preprocess(h_flat, edge_index, Wq, Wk, Wv, Ws)
    res = run_bass_kernel_spmd(nc, in_maps, list(range(N_CORES)))

    alpha_map = np.concatenate(
        [res.results[c]["alpha"] for c in range(N_CORES)], axis=0)
    # [NBLK, NPG(i), BLK(g), DOUT] -> rows (blk, g, i)
    hw_parts = [
        res.results[c]["hw"].reshape(NBLK, NPG, BLK, DOUT)
        .transpose(0, 2, 1, 3).reshape(NPC, DOUT)
        for c in range(N_CORES)
    ]
    h_weighted = np.concatenate(hw_parts, axis=0)
    return alpha_map.reshape(B, NPG), h_weighted.reshape(N, DOUT)


# revision 8
# speedup vs baseline: 3.2982x; 1.1875x over previous
"""Bass/Trainium2 kernel for nn_NodeAttention (PyG TransformerConv-style
edge attention + per-graph scatter softmax), data-parallel over graphs on
8 NeuronCores.

Contract: kernel(**inputs) takes the FULL unsharded inputs (as produced by
reference.setup_inputs()) and returns the full (alpha_map, h_weighted).

Sharding strategy (hardcoded): 512 graphs of 116 nodes -> 64 graphs/core,
processed as 8 blocks x 8 graphs. Wq/Wk/Wv/Ws replicated. Edge softmax and
graph softmax are both graph-local so there is no cross-core communication.

Algorithm per graph g (116 nodes, 8 incoming edges per node):
  Dense scores St[j,i] = k_j . q_i / sqrt(116)  (PE matmul, fp16 in / f32 acc)
  W[j,i] = C[j,i] * exp(St[j,i])  where C counts edges src j -> dst i
           (C=0 masks non-edges; counts handle duplicate edges; the softmax
            max-subtraction is skipped -- scores are ~N(0,1) so exp is safe)
  [msg | denom] = W^T @ [V | 1]   (one matmul per head, ones-column trick)
  hproj = msg / denom + h @ Ws    (fused DVE op, emits row-sum for phase 2)
  Graph softmax over nodes: transpose the per-block e-matrix [116,8] with the
  PE so the node axis lands on the free dim, exp+sum on ACT, normalize, and
  transpose alpha back for the per-node scaling of hproj.

Both heads share one PSUM tile for scores/messages so exp / mask-multiply /
reciprocal run once per graph at [116,232] granularity (ACT per-op overhead
is ~180ns; DVE fp16 SBUF ops hit the 2x perf mode).

h_weighted is written to DRAM in a block-contiguous scratch layout
[8, 116, 8*232] (one 7.4KB run per partition per block -> ~116 DMA
descriptors instead of 928) and transposed to [N, 232] on the host.
"""

import math
import sys

sys.path.insert(0, "/opt/trn_rl_repo")

import numpy as np

from concourse import bacc, mybir, tile
from concourse.bass_utils import run_bass_kernel_spmd
from concourse.masks import make_identity

# Problem geometry (fixed by the nn_NodeAttention_66365834658168 spec)
B = 512           # graphs
NPG = 116         # nodes per graph
HEADS = 2
DH = 116          # head dim
HID = 256
DOUT = HEADS * DH  # 232
DEG = 8
N = B * NPG       # 59392
N_CORES = 8
GPC = B // N_CORES      # 64 graphs per core
NPC = GPC * NPG         # 7424 nodes per core
BLK = 8                 # graphs per block
NBLK = GPC // BLK       # 8 blocks per core
BN = BLK * NPG          # 928 nodes per block

F16 = mybir.dt.float16
F32 = mybir.dt.float32
AF = mybir.ActivationFunctionType
OP = mybir.AluOpType

_CACHE = {}


def _build_module():
    """Build + compile the per-core Bass module (same program on all cores)."""
    nc = bacc.Bacc("TRN2", target_bir_lowering=False, debug=False,
                   num_devices=N_CORES)

    ht_d = nc.dram_tensor("ht", [2, 128, NPC], F16, kind="ExternalInput")
    ct_d = nc.dram_tensor("ct", [NBLK, NPG, BLK * DOUT], F16,
                          kind="ExternalInput")
    wq_d = nc.dram_tensor("wq", [2, 128, DOUT], F16, kind="ExternalInput")
    wk_d = nc.dram_tensor("wk", [2, 128, DOUT], F16, kind="ExternalInput")
    wvs_d = nc.dram_tensor("wvs", [2, 128, 2 * DOUT], F16, kind="ExternalInput")
    alpha_d = nc.dram_tensor("alpha", [GPC, NPG], F32, kind="ExternalOutput")
    # block-contiguous scratch layout; host transposes to [N, 232]
    hw_d = nc.dram_tensor("hw", [NBLK, NPG, BLK * DOUT], F16,
                          kind="ExternalOutput")

    RT = 1.0 / math.sqrt(float(DH))

    with tile.TileContext(nc) as tc:
        with (
            tc.tile_pool(name="const", bufs=1) as constp,
            tc.tile_pool(name="wpool", bufs=1) as wpool,
            tc.tile_pool(name="htp", bufs=2) as htp,
            tc.tile_pool(name="ctp", bufs=2) as ctp,
            tc.tile_pool(name="qkp", bufs=2) as qkp,
            tc.tile_pool(name="vp", bufs=4) as vp,
            tc.tile_pool(name="attp", bufs=4) as attp,
            tc.tile_pool(name="smallp", bufs=24) as smallp,
            tc.tile_pool(name="hwp", bufs=2) as hwp,
            tc.tile_pool(name="ph2", bufs=2) as ph2p,
            tc.tile_pool(name="pj_ps", bufs=2, space="PSUM") as pj_ps,
            tc.tile_pool(name="vs_ps", bufs=2, space="PSUM") as vs_ps,
            tc.tile_pool(name="at_ps", bufs=2, space="PSUM") as at_ps,
        ):
            idt = constp.tile([128, 128], F32)
            make_identity(nc, idt[:])

            wq_sb = wpool.tile([128, 2 * DOUT], F16, tag="wq")
            nc.sync.dma_start(
                wq_sb[:].rearrange("p (k m) -> p k m", k=2),
                wq_d.ap().rearrange("k p m -> p k m"),
            )
            wk_sb = wpool.tile([128, 2 * DOUT], F16, tag="wk")
            nc.sync.dma_start(
                wk_sb[:].rearrange("p (k m) -> p k m", k=2),
                wk_d.ap().rearrange("k p m -> p k m"),
            )
            wvs_sb = wpool.tile([128, 2 * 2 * DOUT], F16, tag="wvs")
            nc.sync.dma_start(
                wvs_sb[:].rearrange("p (k m) -> p k m", k=2),
                wvs_d.ap().rearrange("k p m -> p k m"),
            )

            for b in range(NBLK):
                # ---- loads for this block ----
                ht_sb = htp.tile([128, 2 * BN], F16, tag="ht")
                for k in range(2):
                    nc.sync.dma_start(
                        ht_sb[:, k * BN:(k + 1) * BN],
                        ht_d.ap()[k, :, b * BN:(b + 1) * BN],
                    )
                ct_sb = ctp.tile([NPG, BLK * DOUT], F16, tag="ct")
                nc.sync.dma_start(ct_sb[:], ct_d.ap()[b])

                # ---- Q/K projections (transposed layout [head_d, node]) ----
                qt_sb = qkp.tile([128, 2 * BN], F16, tag="qt")
                kt_sb = qkp.tile([128, 2 * BN], F16, tag="kt")
                for wsb, tsb, eng in ((wq_sb, qt_sb, "v"), (wk_sb, kt_sb, "s")):
                    for h in range(HEADS):
                        for ch in range(2):
                            ps = pj_ps.tile([DH, 464], F32, tag="pj")
                            for k in range(2):
                                nc.tensor.matmul(
                                    ps[:],
                                    lhsT=wsb[:, k * DOUT + h * DH:
                                             k * DOUT + h * DH + DH],
                                    rhs=ht_sb[:, k * BN + ch * 464:
                                              k * BN + ch * 464 + 464],
                                    start=(k == 0), stop=(k == 1),
                                )
                            dst = tsb[:DH, h * BN + ch * 464:
                                      h * BN + ch * 464 + 464]
                            if eng == "v":
                                nc.vector.tensor_copy(dst, ps[:])
                            else:
                                nc.scalar.copy(dst, ps[:])

                hwb = hwp.tile([NPG, BLK * DOUT], F16, tag="hw")
                e_blk = ph2p.tile([NPG, BLK], F32, tag="eblk")

                for gib in range(BLK):
                    g0 = gib * NPG
                    # ---- V and skip (h@Ws) projections, natural layout ----
                    vs = vs_ps.tile([NPG, 2 * DOUT], F32, tag="vs")
                    for k in range(2):
                        nc.tensor.matmul(
                            vs[:],
                            lhsT=ht_sb[:, k * BN + g0:k * BN + g0 + NPG],
                            rhs=wvs_sb[:, k * 2 * DOUT:(k + 1) * 2 * DOUT],
                            start=(k == 0), stop=(k == 1),
                        )
                    # V packed per head with a ones column: [V_h | 1] x2
                    v_sb = vp.tile([NPG, 2 * (DH + 1)], F16, tag="v")
                    v3 = v_sb[:].rearrange("p (h x) -> p h x", h=2)
                    nc.scalar.copy(
                        v3[:, :, 0:DH],
                        vs[:, 0:DOUT].rearrange("p (h d) -> p h d", h=2),
                    )
                    nc.gpsimd.memset(v3[:, :, DH:DH + 1], 1.0)
                    skip_sb = vp.tile([NPG, DOUT], F32, tag="skip")
                    nc.scalar.copy(skip_sb[:], vs[:, DOUT:2 * DOUT])

                    # ---- attention: both heads share one PSUM tile ----
                    st = at_ps.tile([NPG, DOUT], F32, tag="st")
                    for h in range(HEADS):
                        nc.tensor.matmul(
                            st[:, h * DH:(h + 1) * DH],
                            lhsT=kt_sb[:DH, h * BN + g0:h * BN + g0 + NPG],
                            rhs=qt_sb[:DH, h * BN + g0:h * BN + g0 + NPG],
                            start=True, stop=True,
                        )
                    ex = attp.tile([NPG, DOUT], F16, tag="ex")
                    nc.scalar.activation(ex[:], st[:], AF.Exp, scale=RT)
                    wt = attp.tile([NPG, DOUT], F16, tag="wt")
                    nc.vector.tensor_mul(wt[:], ex[:],
                                         ct_sb[:, gib * DOUT:(gib + 1) * DOUT])
                    mg = at_ps.tile([NPG, 2 * (DH + 1)], F32, tag="mg")
                    for h in range(HEADS):
                        nc.tensor.matmul(
                            mg[:, h * (DH + 1):(h + 1) * (DH + 1)],
                            lhsT=wt[:, h * DH:(h + 1) * DH],
                            rhs=v_sb[:, h * (DH + 1):(h + 1) * (DH + 1)],
                            start=True, stop=True,
                        )
                    mg3 = mg[:].rearrange("p (h x) -> p h x", h=2)
                    rec = smallp.tile([NPG, 2], F32, tag="rec")
                    nc.vector.reciprocal(rec[:], mg3[:, :, DH:DH + 1])
                    e_hs = []
                    for h in range(HEADS):
                        e_h = smallp.tile([NPG, 1], F32, tag="eh")
                        nc.vector.scalar_tensor_tensor(
                            out=hwb[:, gib * DOUT + h * DH:
                                    gib * DOUT + h * DH + DH],
                            in0=mg3[:, h, 0:DH],
                            scalar=rec[:, h:h + 1],
                            in1=skip_sb[:, h * DH:(h + 1) * DH],
                            op0=OP.mult, op1=OP.add,
                            accum_out=e_h[:],
                        )
                        e_hs.append(e_h)
                    nc.gpsimd.tensor_add(e_blk[:, gib:gib + 1],
                                         e_hs[0][:], e_hs[1][:])

                # ---- phase 2: per-graph softmax over the 116 nodes ----
                eT = at_ps.tile([BLK, NPG], F32, tag="st")
                nc.tensor.transpose(out=eT[:], in_=e_blk[:],
                                    identity=idt[:NPG, :NPG])
                ee_rows = ph2p.tile([BLK, NPG], F32, tag="ee")
                es = ph2p.tile([BLK, 1], F32, tag="es")
                nc.scalar.activation(ee_rows[:], eT[:], AF.Exp,
                                     scale=1.0 / DOUT, accum_out=es[:])
                rs = ph2p.tile([BLK, 1], F32, tag="rs")
                nc.vector.reciprocal(rs[:], es[:])
                al_rows = ph2p.tile([BLK, NPG], F32, tag="al")
                nc.vector.tensor_scalar_mul(al_rows[:], ee_rows[:], rs[:])
                nc.sync.dma_start(alpha_d.ap()[b * BLK:(b + 1) * BLK, :],
                                  al_rows[:])
                alT = at_ps.tile([NPG, BLK], F32, tag="st")
                nc.tensor.transpose(out=alT[:], in_=al_rows[:],
                                    identity=idt[:BLK, :BLK])
                al_cols = ph2p.tile([NPG, BLK], F32, tag="alc")
                nc.vector.tensor_copy(al_cols[:], alT[:])
                for gib in range(BLK):
                    sl = hwb[:, gib * DOUT:(gib + 1) * DOUT]
                    nc.vector.tensor_scalar_mul(sl, sl,
                                                al_cols[:, gib:gib + 1])
                nc.sync.dma_start(hw_d.ap()[b], hwb[:])

    nc.compile()
    return nc


def _preprocess(h_flat, edge_index, Wq, Wk, Wv, Ws):
    h = np.asarray(h_flat, dtype=np.float32)
    hT16 = np.ascontiguousarray(h.T).astype(np.float16)  # [256, N]

    ei = np.asarray(edge_index, dtype=np.int64)
    src, dst = ei[0], ei[1]
    g = dst // NPG
    i_off = dst - g * NPG
    j_off = src - g * NPG
    cnt = np.bincount(((g * NPG + j_off) * NPG + i_off),
                      minlength=B * NPG * NPG).astype(np.float16)
    cnt = cnt.reshape(B, NPG, NPG)  # [graph, src j, dst i]

    wq16 = np.ascontiguousarray(
        np.asarray(Wq, np.float32).astype(np.float16).reshape(2, 128, DOUT))
    wk16 = np.ascontiguousarray(
        np.asarray(Wk, np.float32).astype(np.float16).reshape(2, 128, DOUT))
    wvs16 = np.ascontiguousarray(
        np.concatenate([np.asarray(Wv, np.float32), np.asarray(Ws, np.float32)],
                       axis=1).astype(np.float16).reshape(2, 128, 2 * DOUT))

    in_maps = []
    for c in range(N_CORES):
        ht_c = np.ascontiguousarray(
            hT16[:, c * NPC:(c + 1) * NPC]).reshape(2, 128, NPC)
        cc = cnt[c * GPC:(c + 1) * GPC].reshape(NBLK, BLK, NPG, NPG)
        # [blk, gib, j, i] -> [blk, j, gib, h, i] with the head axis duplicated
        cc = np.broadcast_to(cc.transpose(0, 2, 1, 3)[:, :, :, None, :],
                             (NBLK, NPG, BLK, HEADS, NPG))
        ct_c = np.ascontiguousarray(cc).reshape(NBLK, NPG, BLK * DOUT)
        in_maps.append({"ht": ht_c, "ct": ct_c,
                        "wq": wq16, "wk": wk16, "wvs": wvs16})
    return in_maps


def kernel(h_flat, edge_index, batch_index, Wq, Wk, Wv, Ws, **_ignored):
    if "nc" not in _CACHE:
        _CACHE["nc"] = _build_module()
    nc = _CACHE["nc"]

    in_maps = _preprocess(h_flat, edge_index, Wq, Wk, Wv, Ws)
    res = run_bass_kernel_spmd(nc, in_maps, list(range(N_CORES)))

    alpha_map = np.concatenate(
        [res.results[c]["alpha"] for c in range(N_CORES)], axis=0)
    # [NBLK, NPG(i), BLK(g), DOUT] -> rows (blk, g, i)
    hw_parts = [
        res.results[c]["hw"].astype(np.float32)
        .reshape(NBLK, NPG, BLK, DOUT)
        .transpose(0, 2, 1, 3).reshape(NPC, DOUT)
        for c in range(N_CORES)
    ]
    h_weighted = np.concatenate(hw_parts, axis=0)
    return alpha_map.reshape(B, NPG), h_weighted.reshape(N, DOUT)
